# revision 2
# baseline (speedup 1.0000x reference)
# kernel.py — Bidirectional masked-GRU-with-predictor on 8 Trainium2 NeuronCores.
#
# Problem (reference.py): B=128, T=1024, H=512
#   per step, per direction:
#     x_in = where(mask, predictor(h), x)            predictor: Linear(H,H)->ReLU->Linear(H,1)->Tanh
#     h    = GRUCell(h, x_in)                        PyTorch gate order (r, z, n)
#   output [B, T, 2H] = concat(fwd hidden states, time-reversed bwd hidden states)
#
# Sharding: 8 cores = 2 directions x 4 batch groups of 32.  All cores run the
# SAME Bass program; per-core data differs (bwd cores get time-reversed x/mask
# and their outputs are flipped back on the host).
#
# On-core layout ("feature-major, chunk-in-free"):
#   h^T kept as [128 partitions = feature%128, (j,b)] where j = feature//128 (4 chunks),
#   b = local batch (32).  Big matmul: stationary = W^T 128x128 blocks (fp16, FWL),
#   moving = h chunks; gates + predictor-hidden land in PSUM feature-major, so the
#   new h is produced directly in the layout the next step's matmul consumes.
#   fp16 matmul inputs + fp32 PSUM accumulate + fp32 vector math:
#   measured emulation error vs fp32 reference: ~7e-4 of output absmax.

import numpy as np

B, T, H = 128, 1024, 512
NCORES = 8
BL = B // 4          # 32: batch per core (4 groups x 2 directions)
KC = H // 128        # 4 contraction chunks
MC = (3 * H + H) // 128  # 16 output chunks (w_hh 12 + p_w1 4)
U_DEF = 8            # time steps per For_i iteration

_cache = {}


def _build_program(t_steps=T, u_steps=U_DEF, bl=BL, n_cores=NCORES):
    import concourse.bacc as bacc
    import concourse.bass as bass
    import concourse.tile as tile
    from concourse import mybir

    f16 = mybir.dt.float16
    f32 = mybir.dt.float32

    nc = bacc.Bacc(
        "TRN2",
        target_bir_lowering=False,
        debug=False,
        enable_asserts=False,
        num_devices=n_cores,
    )

    # ---- DRAM tensors (per-core data; same names on every core) ----
    d_wt = nc.dram_tensor("wt", [128, MC * KC * 128], f16, kind="ExternalInput").ap()
    d_w4 = nc.dram_tensor("w4t", [128, 3 * KC * bl], f16, kind="ExternalInput").ap()
    d_bihn = nc.dram_tensor("bihn", [128, KC * bl], f16, kind="ExternalInput").ap()
    d_bc = nc.dram_tensor("bcols", [4, 4 * 128], f16, kind="ExternalInput").ap()
    d_e4 = nc.dram_tensor("e4", [4, KC * bl], f16, kind="ExternalInput").ap()
    d_on2 = nc.dram_tensor("ones2", [2, 128], f16, kind="ExternalInput").ap()
    d_pw2 = nc.dram_tensor("pw2t", [128, KC], f16, kind="ExternalInput").ap()
    d_pb2 = nc.dram_tensor("pb2", [1, 1], f32, kind="ExternalInput").ap()
    d_a = nc.dram_tensor("a_arr", [t_steps, bl], f16, kind="ExternalInput").ap()
    d_m = nc.dram_tensor("m_arr", [t_steps, bl], f16, kind="ExternalInput").ap()
    d_out = nc.dram_tensor(
        "outl", [t_steps, 128, KC, bl], f16, kind="ExternalOutput"
    ).ap()

    Relu = mybir.ActivationFunctionType.Relu
    Tanh = mybir.ActivationFunctionType.Tanh
    Sigmoid = mybir.ActivationFunctionType.Sigmoid

    with tile.TileContext(nc) as tc:
        import contextlib

        with contextlib.ExitStack() as ctx:
            consts = ctx.enter_context(tc.tile_pool(name="consts", bufs=1))
            psum = ctx.enter_context(tc.tile_pool(name="psum", bufs=1, space="PSUM"))
            work = ctx.enter_context(tc.tile_pool(name="work", bufs=2))
            io = ctx.enter_context(tc.tile_pool(name="io", bufs=2))

            # ---- constant preload ----
            WT = consts.tile([128, MC * KC * 128], f16, tag="WT")
            W4 = consts.tile([128, 3 * KC * bl], f16, tag="W4")
            BIHN = consts.tile([128, KC * bl], f16, tag="BIHN")
            BC = consts.tile([4, 4 * 128], f16, tag="BC")
            E4 = consts.tile([4, KC * bl], f16, tag="E4")
            ON2 = consts.tile([2, 128], f16, tag="ON2")
            PW2 = consts.tile([128, KC], f16, tag="PW2")
            PB2 = consts.tile([1, 1], f32, tag="PB2")
            for dst, src in (
                (WT, d_wt), (W4, d_w4), (BIHN, d_bihn), (BC, d_bc),
                (E4, d_e4), (ON2, d_on2), (PW2, d_pw2), (PB2, d_pb2),
            ):
                nc.sync.dma_start(out=dst, in_=src)

            # persistent ping-pong hidden state, fp16, [128, (j,b)]
            h0 = consts.tile([128, KC * bl], f16, tag="h0")
            h1 = consts.tile([128, KC * bl], f16, tag="h1")
            nc.vector.memset(h0, 0.0)
            nc.vector.memset(h1, 0.0)
            h_tiles = [h0, h1]

            # persistent PSUM accumulators (single-buffered; readers drain early)
            G_r = psum.tile([128, KC * bl], f32, tag="G_r")
            G_z = psum.tile([128, KC * bl], f32, tag="G_z")
            G_n = psum.tile([128, KC * bl], f32, tag="G_n")
            PHp = psum.tile([128, KC * bl], f32, tag="PH")
            XB = psum.tile([128, KC * bl], f32, tag="XB")
            PRD = psum.tile([1, bl], f32, tag="PRD")

            regions = {  # m-chunk -> (psum tile, bias column block index)
                0: (G_r, 0), 1: (G_z, 1), 2: (G_n, 2), 3: (PHp, 3),
            }

            def w_block(m, k):
                bi = m * KC + k
                return WT[:, bi * 128:(bi + 1) * 128]

            def emit_region(g_idx, region, h_cur):
                # bias matmul opens the accumulation (start=True covers the
                # whole region), then 4 m-chunks x 4 k-chunks of W blocks.
                nc.tensor.matmul(
                    region, BC[:, g_idx * 128:(g_idx + 1) * 128], E4,
                    start=True, stop=False, skip_group_check=True,
                )
                base_m = g_idx * KC if g_idx < 3 else 12
                for j in range(KC):
                    m = base_m + j
                    for k in range(KC):
                        nc.tensor.matmul(
                            region[:, j * bl:(j + 1) * bl],
                            w_block(m, k),
                            h_cur[:, k * bl:(k + 1) * bl],
                            start=False,
                            stop=(k == KC - 1),
                            skip_group_check=True,
                        )

            def step(u, h_cur, h_new, S2, MB, t_dyn):
                # --- predictor hidden: PH = p_w1 @ h + p_b1 ---
                emit_region(3, PHp, h_cur)
                relu = work.tile([128, KC * bl], f16, tag="relu")
                nc.scalar.activation(out=relu, in_=PHp, func=Relu)
                # --- pred = tanh(p_w2 @ relu + p_b2) ---
                for k in range(KC):
                    nc.tensor.matmul(
                        PRD, PW2[:, k:k + 1], relu[:, k * bl:(k + 1) * bl],
                        start=(k == 0), stop=(k == KC - 1), skip_group_check=True,
                    )
                pred = work.tile([1, bl], f16, tag="pred")
                nc.scalar.activation(out=pred, in_=PRD, func=Tanh, bias=PB2[:, :])
                # --- x_in = pred*m + x*(1-m):  tmp row of S2 + host-precomputed a row
                nc.vector.tensor_mul(
                    S2[0:1, u * bl:(u + 1) * bl], pred, MB[0:1, u * bl:(u + 1) * bl]
                )
                # --- XB[c,(j,b)] = x_in[b]  (broadcast via K=2 matmul, rhs 4x-tiled)
                s2b = S2[:, u * bl:(u + 1) * bl]
                rhs = bass.AP(
                    tensor=s2b.tensor, offset=s2b.offset,
                    ap=[s2b.ap[0], [0, KC], [1, bl]],
                )
                nc.tensor.matmul(XB, ON2, rhs, start=True, stop=True,
                                 skip_group_check=True)

                # --- recurrent gates ---
                emit_region(0, G_r, h_cur)   # r
                emit_region(2, G_n, h_cur)   # n (bias = b_hh only)
                emit_region(1, G_z, h_cur)   # z

                t_r = work.tile([128, KC * bl], f32, tag="t_r")
                t_z = work.tile([128, KC * bl], f32, tag="t_z")
                t_n = work.tile([128, KC * bl], f32, tag="t_n")
                nc.vector.tensor_mul(t_r, XB, W4[:, 0 * KC * bl:1 * KC * bl])
                nc.vector.tensor_mul(t_z, XB, W4[:, 1 * KC * bl:2 * KC * bl])
                nc.vector.tensor_mul(t_n, XB, W4[:, 2 * KC * bl:3 * KC * bl])

                pre_r = work.tile([128, KC * bl], f32, tag="pre_r")
                nc.vector.tensor_add(pre_r, t_r, G_r)
                r_sb = work.tile([128, KC * bl], f16, tag="r_sb")
                nc.scalar.activation(out=r_sb, in_=pre_r, func=Sigmoid)

                # n = tanh(w4_n*x_in + b_ih_n + r * (gh_n + b_hh_n))
                v_n = work.tile([128, KC * bl], f32, tag="v_n")
                nc.vector.tensor_add(v_n, t_n, BIHN)
                u_n = work.tile([128, KC * bl], f32, tag="u_n")
                nc.vector.tensor_mul(u_n, r_sb, G_n)
                pre_n = work.tile([128, KC * bl], f32, tag="pre_n")
                nc.vector.tensor_add(pre_n, u_n, v_n)
                n_sb = work.tile([128, KC * bl], f16, tag="n_sb")
                nc.scalar.activation(out=n_sb, in_=pre_n, func=Tanh)

                pre_z = work.tile([128, KC * bl], f32, tag="pre_z")
                nc.vector.tensor_add(pre_z, t_z, G_z)
                z_sb = work.tile([128, KC * bl], f16, tag="z_sb")
                nc.scalar.activation(out=z_sb, in_=pre_z, func=Sigmoid)

                # h' = n + z*(h - n)
                d_sb = work.tile([128, KC * bl], f16, tag="d_sb")
                nc.vector.tensor_sub(d_sb, h_cur, n_sb)
                zd = work.tile([128, KC * bl], f16, tag="zd")
                nc.vector.tensor_mul(zd, z_sb, d_sb)
                nc.vector.tensor_add(h_new, n_sb, zd)

                # stream h' out:  outl[t, p, j, b]
                dst = d_out[bass.ds(t_dyn, 1)].rearrange("o p j b -> (o p) j b")
                nc.sync.dma_start(
                    out=dst, in_=h_new.rearrange("p (j b) -> p j b", b=bl)
                )

            n_blocks = t_steps // u_steps
            with tc.For_i(
                0, n_blocks, 1, hint_engines=(mybir.EngineType.PE,)
            ) as iv:
                S2 = io.tile([2, u_steps * bl], f16, tag="S2")
                MB = io.tile([1, u_steps * bl], f16, tag="MB")
                nc.sync.dma_start(
                    out=S2[1:2, :].rearrange("p (u b) -> p u b", b=bl),
                    in_=d_a[bass.ds(iv * u_steps, u_steps)].unsqueeze(0),
                )
                nc.sync.dma_start(
                    out=MB[0:1, :].rearrange("p (u b) -> p u b", b=bl),
                    in_=d_m[bass.ds(iv * u_steps, u_steps)].unsqueeze(0),
                )
                for u in range(u_steps):
                    step(
                        u,
                        h_tiles[u % 2],
                        h_tiles[(u + 1) % 2],
                        S2,
                        MB,
                        iv * u_steps + u,
                    )

    nc.compile()
    return nc


def _prep_core_inputs(inputs, core, t_steps=T, bl=BL):
    """Build the per-core input map (numpy) for core id `core`."""
    f16 = np.float16
    direction = 0 if core < 4 else 1  # 0 fwd, 1 bwd
    bg = core % 4
    sl = slice(bg * bl, (bg + 1) * bl)

    x = np.asarray(inputs["x"], np.float32)[:, :, 0]      # [B, T]
    msk = np.asarray(inputs["mask"]).astype(np.float32)[:, :, 0]
    pfx = "wf" if direction == 0 else "wb"
    w_ih = np.asarray(inputs[f"{pfx}_ih"], np.float32)[:, 0]   # [3H]
    w_hh = np.asarray(inputs[f"{pfx}_hh"], np.float32)         # [3H, H]
    b_ih = np.asarray(inputs[f"b{pfx[1]}_ih"], np.float32)
    b_hh = np.asarray(inputs[f"b{pfx[1]}_hh"], np.float32)
    p_w1 = np.asarray(inputs["p_w1"], np.float32)
    p_b1 = np.asarray(inputs["p_b1"], np.float32)
    p_w2 = np.asarray(inputs["p_w2"], np.float32)
    p_b2 = np.asarray(inputs["p_b2"], np.float32)

    xs = x[sl].T.copy()      # [T, bl]
    ms = msk[sl].T.copy()
    if direction == 1:
        xs = xs[::-1].copy()
        ms = ms[::-1].copy()
    a_arr = (xs * (1.0 - ms)).astype(f16)
    m_arr = ms.astype(f16)

    W = np.concatenate([w_hh, p_w1], axis=0)             # [2048, 512]
    Wr = W.reshape(MC, 128, KC, 128)                     # [m, c, k, p]
    wt = Wr.transpose(3, 0, 2, 1).reshape(128, MC * KC * 128).astype(f16)

    def tile_col(col):  # [512] -> [128, KC*bl], content col[j*128+c] bcast over b
        t = col.reshape(KC, 128).T                       # [128(c), KC(j)]
        return np.broadcast_to(t[:, :, None], (128, KC, bl)).reshape(
            128, KC * bl
        ).astype(f16)

    w4t = np.concatenate(
        [tile_col(w_ih[g * H:(g + 1) * H]) for g in range(3)], axis=1
    )
    bihn = tile_col(b_ih[2 * H:3 * H])

    bias_regions = [
        b_ih[0:H] + b_hh[0:H],          # r
        b_ih[H:2 * H] + b_hh[H:2 * H],  # z
        b_hh[2 * H:3 * H],              # n: b_hh only (b_ih_n added via BIHN)
        p_b1,                           # ph
    ]
    bcols = np.concatenate(
        [br.reshape(KC, 128) for br in bias_regions], axis=1
    ).astype(f16)                                        # [4, 4*128]

    e4 = np.zeros((KC, KC, bl), np.float32)
    for j in range(KC):
        e4[j, j, :] = 1.0
    e4 = e4.reshape(KC, KC * bl).astype(f16)

    ones2 = np.ones((2, 128), f16)
    pw2t = p_w2[0].reshape(KC, 128).T.astype(f16).copy()
    pb2 = p_b2.reshape(1, 1).astype(np.float32)

    return {
        "wt": wt, "w4t": w4t, "bihn": bihn, "bcols": bcols, "e4": e4,
        "ones2": ones2, "pw2t": pw2t, "pb2": pb2,
        "a_arr": a_arr[:t_steps], "m_arr": m_arr[:t_steps],
    }


def _assemble(results, t_steps=T, bl=BL):
    """results: list of 8 per-core dicts with 'outl' [T,128,KC,bl] fp16."""
    out = np.zeros((B, t_steps, 2 * H), np.float32)
    for core in range(NCORES):
        direction = 0 if core < 4 else 1
        bg = core % 4
        arr = np.asarray(results[core]["outl"], np.float16).astype(np.float32)
        # [t, p, j, b] -> [b, t, j, p] -> [b, t, 512]
        arr = arr.transpose(3, 0, 2, 1).reshape(bl, t_steps, H)
        if direction == 1:
            arr = arr[:, ::-1]
        out[bg * bl:(bg + 1) * bl, :, direction * H:(direction + 1) * H] = arr
    return out


def kernel(**inputs):
    from concourse.bass_utils import run_bass_kernel_spmd

    key = (T, U_DEF, BL)
    if key not in _cache:
        _cache[key] = _build_program(T, U_DEF, BL)
    nc = _cache[key]

    in_maps = [_prep_core_inputs(inputs, c) for c in range(NCORES)]
    res = run_bass_kernel_spmd(
        nc, in_maps, core_ids=list(range(NCORES)), trace=False
    )
    return _assemble(res.results)
